# revision 1
# baseline (speedup 1.0000x reference)
"""Trainium2 Bass kernel for nn_EdgeConvolution (gnn_message_passing).

Math
----
Reference (B=2, N=512, C=128, U=128), adj binary {0,1}:
  masked[b,i,j,:]  = adj[b,i,j] * x[b,i,:]
  a_sel[b,i]       = adj[b,i, xidx[b,i]]
  edging[b,i,j,:]  = [ adj*x_i | adj*(a_sel - adj)*x_i ]
                   = adj[b,i,j] * [ x_i | (a_sel_i - 1)*x_i ]        (adj^2 = adj)
  out[b,i,j,:]     = relu(adj*(u_i + (a_sel_i-1)*v_i) + b),  u = x@W1, v = x@W2
So over j there are only two values per (b,i):
  z1_i = relu(u_i + (a_sel_i-1)*v_i + b)   (edges with adj=1, count k_i)
  z0   = relu(b)                            (edges with adj=0, count N-k_i)
  maxp_i   = max(1[k_i>0]*z1_i, 1[k_i<N]*z0)
  n_i      = k_i*1[any z1_i>0] + (N-k_i)*1[any z0>0]
  avgpool_i = [ k_i*x_i | k_i*(a_sel_i-1)*x_i ] / n_i
Per-core slab: 128 of the 1024 (b,i) rows; w/b replicated.

Implementation: raw Bass (no Tile) to minimize semaphore/barrier overhead.
Engines: SP ring DMAs (adj, xidx, b) + out; ACT ring DMAs (x|xT pack, w) +
per-partition-scale multiplies; PE: bias fold (ones x [b|0] accumulated into
x@[W1|W2]) and the b broadcast; DVE: reductions and the main chain; Pool:
iota/cast/[P,1] scalars. `n` is computed by selecting between the two
possible reciprocals so only one op depends on s1 = any(z1>0).
"""

import numpy as np

B, N, C, U = 2, 512, 128, 128
P = 128          # rows (b,i) per core == SBUF partitions
NCORES = 8
OUTF = U + 2 * C  # 384

_CACHE: dict = {}


def _build_nc():
    import concourse.bacc as bacc
    import concourse.bass as bass
    import concourse.mybir as mybir

    f32 = mybir.dt.float32
    i32 = mybir.dt.int32
    Alu = mybir.AluOpType
    AX = mybir.AxisListType.X
    Act = mybir.ActivationFunctionType

    nc = bacc.Bacc("TRN2", target_bir_lowering=False, debug=False,
                   num_devices=NCORES)

    adj_d = nc.dram_tensor("adj", [P, N], f32, kind="ExternalInput")
    xb_d = nc.dram_tensor("xboth", [P, 2 * C], f32, kind="ExternalInput")
    xidx_d = nc.dram_tensor("xidx", [P, 1], i32, kind="ExternalInput")
    w_d = nc.dram_tensor("w", [2 * C, U], f32, kind="ExternalInput")
    b_d = nc.dram_tensor("b", [1, U], f32, kind="ExternalInput")
    out_d = nc.dram_tensor("out", [P, OUTF], f32, kind="ExternalOutput")

    ctx_tensors = [
        ("adj_t", [P, N], f32), ("xb_t", [P, 2 * C], f32),
        ("wcat_t", [P, 2 * U], f32), ("xidx_t", [P, 1], i32),
        ("brow_t", [1, U], f32), ("ones1", [1, P], f32),
        ("iota_f", [P, N], f32), ("xidx_f", [P, 1], f32),
        ("scr", [P, N], f32), ("zcol", [P, 1], f32), ("wscr", [P, 1], f32),
        ("a_sel", [P, 1], f32), ("k", [P, 1], f32), ("asm1", [P, 1], f32),
        ("t_sb", [P, U], f32), ("zz", [P, U], f32), ("zzb", [P, U], f32),
        ("z1", [P, U], f32),
        ("z1sum", [P, 1], f32), ("z0", [P, U], f32), ("z0sum", [P, 1], f32),
        ("s0", [P, 1], f32), ("nk", [P, 1], f32), ("h0", [P, 1], f32),
        ("h1", [P, 1], f32), ("t2", [P, 1], f32),
        ("s1", [P, 1], f32), ("nn", [P, 1], f32), ("rn", [P, 1], f32),
        ("xcat", [P, 2 * C], f32), ("z0h", [P, U], f32),
        ("out_t", [P, OUTF], f32),
    ]

    from contextlib import ExitStack
    with ExitStack() as ctx:
        t = {}
        for name, shape, dt in ctx_tensors:
            t[name] = ctx.enter_context(nc.sbuf_tensor(name, shape, dt))
        mm = ctx.enter_context(nc.psum_tensor("mm", [P, 2 * U], f32))
        bc = ctx.enter_context(nc.psum_tensor("bc", [P, U], f32))

        dadj = ctx.enter_context(nc.semaphore("dadj"))
        didx = ctx.enter_context(nc.semaphore("didx"))
        db = ctx.enter_context(nc.semaphore("db"))
        dxb = ctx.enter_context(nc.semaphore("dxb"))
        dwc = ctx.enter_context(nc.semaphore("dwc"))
        sini = ctx.enter_context(nc.semaphore("sini"))
        spe = ctx.enter_context(nc.semaphore("spe"))
        sdve = ctx.enter_context(nc.semaphore("sdve"))
        spool = ctx.enter_context(nc.semaphore("spool"))
        sact = ctx.enter_context(nc.semaphore("sact"))
        sz0 = ctx.enter_context(nc.semaphore("sz0"))
        sfin = ctx.enter_context(nc.semaphore("sfin"))
        dout = ctx.enter_context(nc.semaphore("dout"))

        block = ctx.enter_context(nc.Block())

        ap = lambda h: h.ap()

        # Self-waits use all-incs-so-far thresholds: completions on one
        # engine can retire out of order, so `>= total` is the only
        # order-independent guarantee that a specific producer finished.

        @block.gpsimd
        def _(pool):
            nc.gpsimd.memset(ap(t["ones1"]), 1.0)
            nc.gpsimd.memset(ap(t["zcol"]), 0.0)
            pool.drain().then_inc(sini, 1)
            nc.gpsimd.iota(ap(t["iota_f"]), pattern=[[1, N]], base=0,
                           channel_multiplier=0,
                           allow_small_or_imprecise_dtypes=True
                           ).then_inc(spool, 1)                        # ->1
            pool.wait_ge(didx, 16)
            nc.gpsimd.tensor_copy(ap(t["xidx_f"]),
                                  ap(t["xidx_t"])).then_inc(spool, 1)  # ->2
            pool.wait_ge(sdve, 1)            # k ready
            nc.gpsimd.tensor_scalar(out=ap(t["nk"]), in0=ap(t["k"]),
                                    scalar1=-1.0, scalar2=float(N),
                                    op0=Alu.mult,
                                    op1=Alu.add).then_inc(spool, 1)    # ->3
            nc.gpsimd.tensor_scalar(out=ap(t["h0"]), in0=ap(t["k"]),
                                    scalar1=float(N), scalar2=None,
                                    op0=Alu.is_lt).then_inc(spool, 1)  # ->4
            nc.gpsimd.tensor_scalar(out=ap(t["h1"]), in0=ap(t["k"]),
                                    scalar1=0.0, scalar2=None,
                                    op0=Alu.is_gt).then_inc(spool, 1)  # ->5
            pool.wait_ge(sz0, 1)             # z0sum ready
            nc.gpsimd.tensor_scalar(out=ap(t["s0"]), in0=ap(t["z0sum"]),
                                    scalar1=0.0, scalar2=None,
                                    op0=Alu.is_gt).then_inc(spool, 1)  # ->6
            pool.wait_ge(spool, 6)           # nk + s0 visible (all 6)
            nc.gpsimd.tensor_mul(ap(t["t2"]), ap(t["nk"]),
                                 ap(t["s0"])).then_inc(spool, 1)       # ->7

        @block.sync
        def _(sync):
            sync.dma_start(ap(t["adj_t"]), adj_d.ap()).then_inc(dadj, 16)
            sync.dma_start(ap(t["brow_t"]), b_d.ap()).then_inc(db, 16)
            sync.dma_start(ap(t["xidx_t"]), xidx_d.ap()).then_inc(didx, 16)
            sync.wait_ge(sfin, 2)
            sync.dma_start(out_d.ap(), ap(t["out_t"])).then_inc(dout, 16)
            sync.wait_ge(dout, 16)

        @block.scalar
        def _(act):
            act.dma_start(ap(t["xb_t"]), xb_d.ap()).then_inc(dxb, 16)
            act.dma_start(
                t["wcat_t"].ap().rearrange("p (s u) -> p s u", s=2),
                w_d.ap().rearrange("(s c) u -> c s u", s=2),
            ).then_inc(dwc, 16)
            act.wait_ge(sini, 1)
            # warm the activation table off the critical path
            nc.scalar.activation(out=ap(t["wscr"]), in_=ap(t["zcol"]),
                                 func=Act.Relu, bias=t["zcol"].ap()[:, 0:1])
            act.wait_ge(spe, 1)              # bc = ones x b broadcast done
            nc.scalar.activation(out=ap(t["z0"]), in_=bc.ap(), func=Act.Relu,
                                 bias=t["zcol"].ap()[:, 0:1],
                                 accum_out=t["z0sum"].ap()[:, 0:1]
                                 ).then_inc(sz0, 1)
            act.wait_ge(dxb, 16)
            act.wait_ge(sdve, 1)             # k
            nc.scalar.activation(out=t["xcat"].ap()[:, 0:C],
                                 in_=t["xb_t"].ap()[:, 0:C], func=Act.Copy,
                                 scale=t["k"].ap()[:, 0:1]
                                 ).then_inc(sact, 1)                   # ->1
            act.wait_ge(sdve, 3)             # asm1
            act.wait_ge(sact, 1)             # xk visible (self)
            nc.scalar.activation(out=t["xcat"].ap()[:, C:2 * C],
                                 in_=t["xcat"].ap()[:, 0:C], func=Act.Copy,
                                 scale=t["asm1"].ap()[:, 0:1]
                                 ).then_inc(sact, 1)                   # ->2
            act.wait_ge(spool, 5)            # h0 (all of iota..h1)
            nc.scalar.activation(out=ap(t["z0h"]), in_=ap(t["z0"]),
                                 func=Act.Copy, scale=t["h0"].ap()[:, 0:1]
                                 ).then_inc(sact, 1)                   # ->3
            act.wait_ge(sdve, 10)            # rn
            act.wait_ge(sact, 3)             # xcat fully visible
            nc.scalar.activation(out=t["out_t"].ap()[:, U:OUTF],
                                 in_=ap(t["xcat"]), func=Act.Copy,
                                 scale=t["rn"].ap()[:, 0:1]
                                 ).then_inc(sfin, 1)

        @block.tensor
        def _(pe):
            pe.wait_ge(sini, 1)              # ones1 ready
            pe.wait_ge(db, 16)               # b landed
            nc.tensor.matmul(bc.ap(), lhsT=t["ones1"].ap(),
                             rhs=ap(t["brow_t"]), start=True,
                             stop=True).then_inc(spe, 1)    # ->1 (bc ready)
            pe.wait_ge(dxb, 16)
            pe.wait_ge(dwc, 16)
            nc.tensor.matmul(mm.ap(), lhsT=t["xb_t"].ap()[:, C:2 * C],
                             rhs=t["wcat_t"].ap(), start=True,
                             stop=True).then_inc(spe, 1)    # ->2 (mm ready)

        @block.vector
        def _(dve):
            dve.wait_ge(dadj, 16)
            nc.vector.reduce_sum(ap(t["k"]), ap(t["adj_t"]),
                                 axis=AX).then_inc(sdve, 1)            # ->1
            dve.wait_ge(spool, 2)            # iota + xidx_f
            nc.vector.scalar_tensor_tensor(
                out=ap(t["scr"]), in0=ap(t["iota_f"]),
                scalar=t["xidx_f"].ap()[:, 0:1], in1=ap(t["adj_t"]),
                op0=Alu.is_equal, op1=Alu.mult,
                accum_out=t["a_sel"].ap()[:, 0:1]).then_inc(sdve, 1)   # ->2
            dve.wait_ge(sdve, 2)             # a_sel accum lands async
            nc.vector.tensor_scalar(out=ap(t["asm1"]), in0=ap(t["a_sel"]),
                                    scalar1=-1.0, scalar2=None,
                                    op0=Alu.add).then_inc(sdve, 1)     # ->3
            dve.wait_ge(spe, 2)              # mm = [u | v]
            dve.wait_ge(sdve, 3)             # asm1 visible
            nc.vector.tensor_scalar(out=ap(t["t_sb"]),
                                    in0=mm.ap()[:, U:2 * U],
                                    scalar1=t["asm1"].ap()[:, 0:1],
                                    scalar2=None,
                                    op0=Alu.mult).then_inc(sdve, 1)    # ->4
            dve.wait_ge(sdve, 4)             # t_sb visible
            nc.vector.tensor_add(ap(t["zz"]), ap(t["t_sb"]),
                                 mm.ap()[:, 0:U]).then_inc(sdve, 1)    # ->5
            dve.wait_ge(sdve, 5)             # zz visible
            dve.wait_ge(spe, 2)              # bc ready
            nc.vector.tensor_add(ap(t["zzb"]), ap(t["zz"]),
                                 bc.ap()).then_inc(sdve, 1)            # ->6
            dve.wait_ge(sdve, 6)             # zzb visible
            nc.vector.tensor_scalar(out=ap(t["z1"]), in0=ap(t["zzb"]),
                                    scalar1=0.0, scalar2=None, op0=Alu.max,
                                    op1=Alu.add,
                                    accum_out=t["z1sum"].ap()[:, 0:1]
                                    ).then_inc(sdve, 1)                # ->7
            dve.wait_ge(sdve, 7)             # z1sum accum landed
            nc.vector.tensor_scalar(out=ap(t["s1"]), in0=ap(t["z1sum"]),
                                    scalar1=0.0, scalar2=None,
                                    op0=Alu.is_gt).then_inc(sdve, 1)   # ->8
            dve.wait_ge(spool, 7)            # t2
            dve.wait_ge(sdve, 8)             # s1 visible
            nc.vector.scalar_tensor_tensor(
                out=ap(t["nn"]), in0=ap(t["k"]),
                scalar=t["s1"].ap()[:, 0:1], in1=ap(t["t2"]),
                op0=Alu.mult, op1=Alu.add).then_inc(sdve, 1)           # ->9
            dve.wait_ge(sdve, 9)             # nn visible
            nc.vector.reciprocal(ap(t["rn"]),
                                 ap(t["nn"])).then_inc(sdve, 1)        # ->10
            dve.wait_ge(sact, 3)             # z0h
            nc.vector.scalar_tensor_tensor(
                out=t["out_t"].ap()[:, 0:U], in0=ap(t["z1"]),
                scalar=t["h1"].ap()[:, 0:1], in1=ap(t["z0h"]),
                op0=Alu.mult, op1=Alu.max).then_inc(sfin, 1)

    nc.compile()
    return nc


def get_nc():
    if "nc" not in _CACHE:
        _CACHE["nc"] = _build_nc()
    return _CACHE["nc"]


def make_in_maps(inputs, adj_matrix, xidx, w, b):
    """Shard full inputs into per-core input maps (128 (b,i) rows per core)."""
    x_flat = np.asarray(inputs, dtype=np.float32).reshape(B * N, C)
    adj_flat = np.ascontiguousarray(
        np.asarray(adj_matrix, dtype=np.float32).reshape(B * N, N))
    xidx_flat = np.ascontiguousarray(
        np.asarray(xidx, dtype=np.int32).reshape(B * N, 1))
    w_full = np.ascontiguousarray(np.asarray(w, dtype=np.float32)[0])
    b_full = np.ascontiguousarray(
        np.asarray(b, dtype=np.float32).reshape(1, U))

    in_maps = []
    for c in range(NCORES):
        rows = slice(c * P, (c + 1) * P)
        x_slab = x_flat[rows]
        in_maps.append({
            "adj": adj_flat[rows],
            "xboth": np.ascontiguousarray(
                np.concatenate([x_slab, x_slab.T], axis=1)),
            "xidx": xidx_flat[rows],
            "w": w_full,
            "b": b_full,
        })
    return in_maps


def kernel(inputs, adj_matrix, xidx, w, b, _trace=False):
    from concourse.bass_utils import run_bass_kernel_spmd

    nc = get_nc()
    in_maps = make_in_maps(inputs, adj_matrix, xidx, w, b)
    res = run_bass_kernel_spmd(nc, in_maps, list(range(NCORES)),
                               trace=_trace)
    out = np.concatenate([res.results[c]["out"] for c in range(NCORES)],
                         axis=0)
    out = out.reshape(B, N, OUTF).astype(np.float32)
    if _trace:
        _CACHE["last_results"] = res
    return out

